# revision 32
# baseline (speedup 1.0000x reference)
"""Trainium2 Bass kernel for BILSTM_CRF_Span_Attr_Tail (segment_reduce).

Computes, for full inputs (B=64, S=512, H=768):
  attr_h' = segment-mean-scatter(attr_hiddens, span_labels)   (mean over [last B .. j] at E positions)
  logits_span = span_hiddens @ W_span + b_span                 [B, S, 5]
  logits_attr = attr_h'      @ W_attr + b_attr                 [B, S, 13]

Sharding: data-parallel over batch across 8 NeuronCores (8 rows/core),
weights replicated; no cross-core communication.

Per-core algorithm (all matmuls in float32r, ~1.5e-4 rel err):
  1. Hiddens stream in naturally as [128 tokens, 768] tiles and are
     PE-transposed (128x128 blocks, via identity) into [h, tokens] PSUM
     tiles -- the layout both logits matmuls need as the moving operand.
  2. Segmented cumsum of transposed attr along tokens in ONE DVE
     instruction per [128, 512] tile:  state = r_t*state + x_t  with
     r = 0 at B-labeled positions (tensor_tensor_scan, mult/add).
  3. l_span  = W_span.T @ spanT   (PSUM accumulation over 6 h-chunks)
     l_seg   = W_attr.T @ seg
  4. Column masks commute through the matmul, and x_t = seg_t - r_t*seg_{t-1},
     so the blended attr logits are reconstructed on tiny [13, 512] tiles:
       logits_attr[k,t] = u_t*l_seg[k,t] - v_t*l_seg[k,t-1]
     with u = (1-e) + e/count, v = r*(1-e) -- no per-chunk masking, no
     PSUM->SBUF copy of the attr stream at all.
  5. Label rows (r, u, v) are computed on device; r is broadcast across
     partitions with a PE ones-column outer product (the DMA path would
     queue behind the streaming loads), u/v via tiny DMA broadcasts.

DMA ring discipline: the Sync HWDGE ring carries ONLY the streaming
hidden-state loads (FIFO rings stall behind dependency-blocked heads);
everything else (setup, label rows, broadcasts, output stores) issues
from the GpSimd SWDGE queue.

Measured on trn2 (8 cores): ~118-122us NTFF exec, vs ~70us HBM-stream
floor for the 25.2MB/core of input; rel err ~3.8e-4 vs the fp64-ish
reference (float32r matmul rounding + shift-reconstruction).
"""

import numpy as np

B, S, H = 64, 512, 768
NS, NA = 5, 13
NCORES = 8
BP = B // NCORES  # batch rows per core
B_ID, E_ID = 1, 3

_PROGRAM_CACHE = {}


def build_program():
    """Build + compile the per-core Bass/Tile program (cached)."""
    if "nc" in _PROGRAM_CACHE:
        return _PROGRAM_CACHE["nc"]

    import concourse.bacc as bacc
    import concourse.mybir as mybir
    from concourse.tile import TileContext
    from concourse.masks import make_identity

    f32 = mybir.dt.float32
    f32r = mybir.dt.float32r
    Alu = mybir.AluOpType
    HC = H // 128  # 6 h-chunks
    TC = S // 128  # 4 token-chunks

    nc = bacc.Bacc("TRN2", target_bir_lowering=False, debug=False, num_devices=NCORES)

    labels = nc.dram_tensor("labels", [BP, S], f32, kind="ExternalInput").ap()
    span_h = nc.dram_tensor("span_h", [BP, S, H], f32r, kind="ExternalInput").ap()
    attr_h = nc.dram_tensor("attr_h", [BP, S, H], f32r, kind="ExternalInput").ap()
    w_span = nc.dram_tensor("w_span", [H, NS], f32r, kind="ExternalInput").ap()
    b_span = nc.dram_tensor("b_span", [NS], f32, kind="ExternalInput").ap()
    w_attr = nc.dram_tensor("w_attr", [H, NA], f32r, kind="ExternalInput").ap()
    b_attr = nc.dram_tensor("b_attr", [NA], f32, kind="ExternalInput").ap()
    out_spanT = nc.dram_tensor("out_spanT", [BP, NS, S], f32, kind="ExternalOutput").ap()
    out_attrT = nc.dram_tensor("out_attrT", [BP, NA, S], f32, kind="ExternalOutput").ap()

    with TileContext(nc) as tc:
        with (
            tc.tile_pool(name="consts", bufs=1) as consts,
            tc.tile_pool(name="dram", bufs=1, space="DRAM") as dpool,
            tc.tile_pool(name="nat", bufs=12) as natp,
            tc.tile_pool(name="bcast", bufs=BP) as bcp,
            tc.tile_pool(name="bcast2", bufs=3) as bcp2,
            tc.tile_pool(name="tsb", bufs=3) as tsb,
            tc.tile_pool(name="outp", bufs=2) as outp,
            tc.tile_pool(name="tpps", bufs=5, space="PSUM") as tpps,
            tc.tile_pool(name="lps", bufs=2, space="PSUM") as lps,
            tc.tile_pool(name="lsp1", bufs=1, space="PSUM") as lsp1,
        ):
            ident_f = consts.tile([128, 128], f32)
            make_identity(nc, ident_f)
            ident = consts.tile([128, 128], f32r)
            nc.vector.tensor_copy(ident, ident_f)
            ones_f = consts.tile([1, 128], f32)
            nc.vector.memset(ones_f, 1.0)
            ones_r = consts.tile([1, 128], f32r)
            nc.vector.tensor_copy(ones_r, ones_f)

            lab = consts.tile([BP, S], f32)
            nc.gpsimd.dma_start(out=lab, in_=labels)
            lab1 = consts.tile([1, BP * S], f32)
            nc.gpsimd.dma_start(out=lab1, in_=labels.rearrange("(o b) s -> o (b s)", o=1))
            w_span_sb = consts.tile([128, HC, NS], f32r)
            nc.gpsimd.dma_start(out=w_span_sb, in_=w_span.rearrange("(c p) k -> p c k", p=128))
            w_attr_sb = consts.tile([128, HC, NA], f32r)
            nc.gpsimd.dma_start(out=w_attr_sb, in_=w_attr.rearrange("(c p) k -> p c k", p=128))
            bs_sb = consts.tile([NS, 1], f32)
            nc.gpsimd.dma_start(out=bs_sb, in_=b_span.rearrange("(k o) -> k o", o=1))
            ba_sb = consts.tile([NA, 1], f32)
            nc.gpsimd.dma_start(out=ba_sb, in_=b_attr.rearrange("(k o) -> k o", o=1))

            # ---- label math (all [BP, S], tokens on the free dim) ----
            isb = consts.tile([BP, S], f32)
            nc.vector.tensor_scalar(isb, lab, float(B_ID), None, Alu.is_equal)
            rows = dpool.tile([2, BP, S], f32)
            emask = consts.tile([BP, S], f32)
            nc.vector.tensor_scalar(emask, lab, float(E_ID), None, Alu.is_equal)
            dmask = consts.tile([BP, S], f32)  # 1 - e: keep original attr cols
            nc.vector.tensor_scalar(dmask, emask, -1.0, 1.0, Alu.mult, Alu.add)
            idx1 = consts.tile([BP, S], f32)  # 1..S per partition
            nc.gpsimd.iota(idx1, pattern=[[1, S]], base=1, channel_multiplier=0,
                           allow_small_or_imprecise_dtypes=True)
            bidx1 = consts.tile([BP, S], f32)  # j+1 at B positions else 0
            nc.vector.tensor_mul(bidx1, idx1, isb)
            lastb1 = consts.tile([BP, S], f32)  # (last B pos)+1, 0 if none
            nc.vector.tensor_tensor_scan(lastb1, bidx1, bidx1, 0.0, Alu.max, Alu.max)
            start_t = consts.tile([BP, S], f32)  # segment start = max(lastb-1, 0)
            nc.vector.tensor_scalar(start_t, lastb1, -1.0, 0.0, Alu.add, Alu.max)
            cnt = consts.tile([BP, S], f32)  # j - start + 1
            nc.vector.tensor_sub(cnt, idx1, start_t)
            invc = consts.tile([BP, S], f32)
            nc.vector.reciprocal(invc, cnt)
            icm = consts.tile([BP, S], f32)   # e/count: mean scale at E, 0 elsewhere
            nc.vector.tensor_mul(icm, invc, emask)
            # logits_attr[k,t] = u_t*(W.T@seg)[k,t] - v_t*(W.T@seg)[k,t-1]
            # with u = (1-e) + e/count, v = r*(1-e)  (x_t = seg_t - r_t*seg_{t-1})
            urow = consts.tile([BP, S], f32)
            nc.vector.tensor_add(urow, dmask, icm)
            rmask8 = consts.tile([BP, S], f32)
            nc.vector.tensor_scalar(rmask8, lab, float(B_ID), None, Alu.not_equal)
            vrow = consts.tile([BP, S], f32)
            nc.vector.tensor_mul(vrow, rmask8, dmask)

            # rB broadcasts gate the first scans: build them on the PE
            # (ones-column outer product), which is idle early -- the DMA
            # path would queue behind the streaming input loads.
            rBs = []
            for b in range(BP):
                rm1 = bcp2.tile([1, S], f32r, tag="rm1")  # row's reset mask at partition 0
                nc.vector.tensor_scalar(rm1, lab1[0:1, b * S:(b + 1) * S], float(B_ID), None, Alu.not_equal)
                rb_ps = tpps.tile([128, S], f32, tag="tp")
                nc.tensor.matmul(rb_ps, ones_r, rm1, start=True, stop=True)
                rB = bcp.tile([128, S], f32r, tag="rB")
                nc.scalar.copy(rB, rb_ps)
                rBs.append(rB)
            nc.gpsimd.dma_start(out=rows[0], in_=urow)
            nc.gpsimd.dma_start(out=rows[1], in_=vrow)
            bcasts = []
            for b in range(BP):
                uB = bcp2.tile([NA, S], f32, tag="uB")
                nc.gpsimd.dma_start(out=uB, in_=rows[0, b:b + 1, :].to_broadcast([NA, S]))
                vB = bcp2.tile([NA, S], f32, tag="vB")
                nc.gpsimd.dma_start(out=vB, in_=rows[1, b:b + 1, :].to_broadcast([NA, S]))
                bcasts.append((rBs[b], uB, vB))

            for b in range(BP):
                rB, uB, vB = bcasts[b]

                attr_nat, span_nat = [], []
                for t in range(TC):
                    a = natp.tile([128, H], f32r, tag="attr_nat")
                    nc.sync.dma_start(out=a, in_=attr_h[b, t * 128:(t + 1) * 128, :])
                    attr_nat.append(a)
                    s = natp.tile([128, H], f32r, tag="span_nat")
                    nc.sync.dma_start(out=s, in_=span_h[b, t * 128:(t + 1) * 128, :])
                    span_nat.append(s)

                l_span = lsp1.tile([NS, S], f32, tag="lspan")
                l_attr2 = lps.tile([NA, S], f32, tag="lattr2")  # W.T @ segsums

                for c in range(HC):
                    hs = slice(c * 128, (c + 1) * 128)
                    # attr transposes first: they gate the scan (the DVE is the
                    # pacing engine), span transposes only gate an ACT copy.
                    aT_ps = tpps.tile([128, S], f32r, tag="tp")
                    for t in range(TC):
                        nc.tensor.transpose(aT_ps[:, t * 128:(t + 1) * 128],
                                            attr_nat[t][:, hs], ident)

                    # segmented cumsum along tokens: state = r*state + x
                    seg = tsb.tile([128, S], f32r, tag="seg")
                    nc.vector.tensor_tensor_scan(seg, rB, aT_ps, 0.0, Alu.mult, Alu.add)

                    sT_ps = tpps.tile([128, S], f32r, tag="tp")
                    for t in range(TC):
                        nc.tensor.transpose(sT_ps[:, t * 128:(t + 1) * 128],
                                            span_nat[t][:, hs], ident)
                    sT_sb = tsb.tile([128, S], f32r, tag="sT")
                    nc.scalar.copy(sT_sb, sT_ps)

                    nc.tensor.matmul(l_attr2, w_attr_sb[:, c, :], seg,
                                     start=(c == 0), stop=(c == HC - 1))
                    nc.tensor.matmul(l_span, w_span_sb[:, c, :], sT_sb,
                                     start=(c == 0), stop=(c == HC - 1))

                oS = outp.tile([NS, S], f32, tag="oS")
                nc.scalar.activation(oS, l_span, mybir.ActivationFunctionType.Identity,
                                     bias=bs_sb, scale=1.0)
                nc.gpsimd.dma_start(out=out_spanT[b], in_=oS)
                m1 = outp.tile([NA, S], f32, tag="m1")
                nc.vector.tensor_mul(m1, l_attr2, uB)
                m2 = outp.tile([NA, S], f32, tag="m2")
                nc.vector.tensor_mul(m2[:, 1:S], l_attr2[:, 0:S - 1], vB[:, 1:S])
                nc.vector.tensor_sub(m1[:, 1:S], m1[:, 1:S], m2[:, 1:S])
                oA = outp.tile([NA, S], f32, tag="oA")
                nc.scalar.activation(oA, m1, mybir.ActivationFunctionType.Identity,
                                     bias=ba_sb, scale=1.0)
                nc.gpsimd.dma_start(out=out_attrT[b], in_=oA)

    nc.compile()
    _PROGRAM_CACHE["nc"] = nc
    return nc


def make_in_maps(span_labels, span_hiddens, attr_hiddens, W_span, b_span, W_attr, b_attr):
    labels_f = np.asarray(span_labels).astype(np.float32)
    span_f = np.ascontiguousarray(np.asarray(span_hiddens, dtype=np.float32))
    attr_f = np.ascontiguousarray(np.asarray(attr_hiddens, dtype=np.float32))
    ws = np.ascontiguousarray(np.asarray(W_span, dtype=np.float32))
    bs = np.ascontiguousarray(np.asarray(b_span, dtype=np.float32))
    wa = np.ascontiguousarray(np.asarray(W_attr, dtype=np.float32))
    ba = np.ascontiguousarray(np.asarray(b_attr, dtype=np.float32))
    in_maps = []
    for i in range(NCORES):
        sl = slice(i * BP, (i + 1) * BP)
        in_maps.append({
            "labels": np.ascontiguousarray(labels_f[sl]),
            "span_h": np.ascontiguousarray(span_f[sl]),
            "attr_h": np.ascontiguousarray(attr_f[sl]),
            "w_span": ws, "b_span": bs, "w_attr": wa, "b_attr": ba,
        })
    return in_maps


def assemble(results):
    lspanT = np.stack([r["out_spanT"] for r in results])  # [NCORES, BP, NS, S]
    lattrT = np.stack([r["out_attrT"] for r in results])
    logits_span = lspanT.reshape(B, NS, S).transpose(0, 2, 1).copy()
    logits_attr = lattrT.reshape(B, NA, S).transpose(0, 2, 1).copy()
    return logits_span, logits_attr


def run(inputs, trace=False, tmpdir=None):
    """Run on the 8 NeuronCores. Returns ((logits_span, logits_attr), exec_time_ns)."""
    from concourse.bass_utils import run_bass_kernel_spmd

    nc = build_program()
    in_maps = make_in_maps(**{k: inputs[k] for k in (
        "span_labels", "span_hiddens", "attr_hiddens",
        "W_span", "b_span", "W_attr", "b_attr")})
    if trace:
        _register_ntff_hook()
    res = run_bass_kernel_spmd(nc, in_maps, list(range(NCORES)), trace=trace,
                               tmpdir=tmpdir)
    return assemble(res.results), res.exec_time_ns


def _register_ntff_hook():
    """The agent image lacks antenv.axon_hooks; synthesize it so trace=True
    can reach the axon NTFF profiler. No-op if already registered."""
    import sys, types
    try:
        import antenv.axon_hooks  # noqa: F401
        return
    except ImportError:
        pass
    mod = types.ModuleType("antenv.axon_hooks")
    _h = [None]
    mod.set_axon_ntff_profile_hook = lambda h: _h.__setitem__(0, h)
    mod.get_axon_ntff_profile_hook = lambda: _h[0]
    sys.modules["antenv.axon_hooks"] = mod
    try:
        from trn_agent_boot.trn_boot import _ntff_profile_via_ctypes
        mod.set_axon_ntff_profile_hook(_ntff_profile_via_ctypes("/opt/axon/libaxon_pjrt.so"))
    except Exception:
        pass
    import concourse.bass_utils as bass_utils
    bass_utils.upload_artifacts = lambda tmpdir: "local://" + str(tmpdir)


def kernel(span_labels, span_hiddens, attr_hiddens, W_span, b_span, W_attr, b_attr):
    (logits_span, logits_attr), _ = run({
        "span_labels": span_labels, "span_hiddens": span_hiddens,
        "attr_hiddens": attr_hiddens, "W_span": W_span, "b_span": b_span,
        "W_attr": W_attr, "b_attr": b_attr,
    })
    return logits_span, logits_attr


# revision 35
# speedup vs baseline: 1.0220x; 1.0220x over previous
"""Trainium2 Bass kernel for BILSTM_CRF_Span_Attr_Tail (segment_reduce).

Computes, for full inputs (B=64, S=512, H=768):
  attr_h' = segment-mean-scatter(attr_hiddens, span_labels)   (mean over [last B .. j] at E positions)
  logits_span = span_hiddens @ W_span + b_span                 [B, S, 5]
  logits_attr = attr_h'      @ W_attr + b_attr                 [B, S, 13]

Sharding: data-parallel over batch across 8 NeuronCores (8 rows/core),
weights replicated; no cross-core communication.

Per-core algorithm (all matmuls in float32r, ~1.5e-4 rel err):
  1. Hiddens stream in naturally as [128 tokens, 768] tiles and are
     PE-transposed (128x128 blocks, via identity) into [h, tokens] PSUM
     tiles -- the layout both logits matmuls need as the moving operand.
  2. Segmented cumsum of transposed attr along tokens in ONE DVE
     instruction per [128, 512] tile:  state = r_t*state + x_t  with
     r = 0 at B-labeled positions (tensor_tensor_scan, mult/add).
  3. l_span  = W_span.T @ spanT   (PSUM accumulation over 6 h-chunks)
     l_seg   = W_attr.T @ seg
  4. Column masks commute through the matmul, and x_t = seg_t - r_t*seg_{t-1},
     so the blended attr logits are reconstructed on tiny [13, 512] tiles:
       logits_attr[k,t] = u_t*l_seg[k,t] - v_t*l_seg[k,t-1]
     with u = (1-e) + e/count, v = r*(1-e) -- no per-chunk masking, no
     PSUM->SBUF copy of the attr stream at all.
  5. Label rows (r, u, v) are computed on device; r is broadcast across
     partitions with a PE ones-column outer product (the DMA path would
     queue behind the streaming loads), u/v via tiny DMA broadcasts.

DMA ring discipline: the Sync HWDGE ring carries ONLY the streaming
hidden-state loads (FIFO rings stall behind dependency-blocked heads);
everything else (setup, label rows, broadcasts, output stores) issues
from the GpSimd SWDGE queue.

Measured on trn2 (8 cores): ~118-122us NTFF exec, vs ~70us HBM-stream
floor for the 25.2MB/core of input; rel err ~3.8e-4 vs the fp64-ish
reference (float32r matmul rounding + shift-reconstruction).
"""

import numpy as np

B, S, H = 64, 512, 768
NS, NA = 5, 13
NCORES = 8
BP = B // NCORES  # batch rows per core
B_ID, E_ID = 1, 3

_PROGRAM_CACHE = {}


def build_program():
    """Build + compile the per-core Bass/Tile program (cached)."""
    if "nc" in _PROGRAM_CACHE:
        return _PROGRAM_CACHE["nc"]

    import concourse.bacc as bacc
    import concourse.mybir as mybir
    from concourse.tile import TileContext
    from concourse.masks import make_identity

    f32 = mybir.dt.float32
    f32r = mybir.dt.float32r
    Alu = mybir.AluOpType
    HC = H // 128  # 6 h-chunks
    TC = S // 128  # 4 token-chunks

    nc = bacc.Bacc("TRN2", target_bir_lowering=False, debug=False, num_devices=NCORES)

    labels = nc.dram_tensor("labels", [BP, S], f32, kind="ExternalInput").ap()
    span_h = nc.dram_tensor("span_h", [BP, S, H], f32r, kind="ExternalInput").ap()
    attr_h = nc.dram_tensor("attr_h", [BP, S, H], f32r, kind="ExternalInput").ap()
    w_span = nc.dram_tensor("w_span", [H, NS], f32r, kind="ExternalInput").ap()
    b_span = nc.dram_tensor("b_span", [NS], f32, kind="ExternalInput").ap()
    w_attr = nc.dram_tensor("w_attr", [H, NA], f32r, kind="ExternalInput").ap()
    b_attr = nc.dram_tensor("b_attr", [NA], f32, kind="ExternalInput").ap()
    out_spanT = nc.dram_tensor("out_spanT", [BP, NS, S], f32, kind="ExternalOutput").ap()
    out_attrT = nc.dram_tensor("out_attrT", [BP, NA, S], f32, kind="ExternalOutput").ap()

    with TileContext(nc) as tc:
        with (
            tc.tile_pool(name="consts", bufs=1) as consts,
            tc.tile_pool(name="dram", bufs=1, space="DRAM") as dpool,
            tc.tile_pool(name="nat", bufs=12) as natp,
            tc.tile_pool(name="bcast", bufs=BP) as bcp,
            tc.tile_pool(name="bcast2", bufs=3) as bcp2,
            tc.tile_pool(name="tsb", bufs=3) as tsb,
            tc.tile_pool(name="outp", bufs=2) as outp,
            tc.tile_pool(name="tpps", bufs=6, space="PSUM") as tpps,
            tc.tile_pool(name="lps", bufs=2, space="PSUM") as lps,
            tc.tile_pool(name="lsp1", bufs=1, space="PSUM") as lsp1,
        ):
            ident_f = consts.tile([128, 128], f32)
            make_identity(nc, ident_f)
            ident = consts.tile([128, 128], f32r)
            nc.vector.tensor_copy(ident, ident_f)
            ones_f = consts.tile([1, 128], f32)
            nc.vector.memset(ones_f, 1.0)
            ones_r = consts.tile([1, 128], f32r)
            nc.vector.tensor_copy(ones_r, ones_f)

            lab = consts.tile([BP, S], f32)
            nc.gpsimd.dma_start(out=lab, in_=labels)
            lab1 = consts.tile([1, BP * S], f32)
            nc.gpsimd.dma_start(out=lab1, in_=labels.rearrange("(o b) s -> o (b s)", o=1))
            w_span_sb = consts.tile([128, HC, NS], f32r)
            nc.gpsimd.dma_start(out=w_span_sb, in_=w_span.rearrange("(c p) k -> p c k", p=128))
            w_attr_sb = consts.tile([128, HC, NA], f32r)
            nc.gpsimd.dma_start(out=w_attr_sb, in_=w_attr.rearrange("(c p) k -> p c k", p=128))
            bs_sb = consts.tile([NS, 1], f32)
            nc.gpsimd.dma_start(out=bs_sb, in_=b_span.rearrange("(k o) -> k o", o=1))
            ba_sb = consts.tile([NA, 1], f32)
            nc.gpsimd.dma_start(out=ba_sb, in_=b_attr.rearrange("(k o) -> k o", o=1))

            # ---- label math (all [BP, S], tokens on the free dim) ----
            isb = consts.tile([BP, S], f32)
            nc.vector.tensor_scalar(isb, lab, float(B_ID), None, Alu.is_equal)
            rows = dpool.tile([2, BP, S], f32)
            emask = consts.tile([BP, S], f32)
            nc.vector.tensor_scalar(emask, lab, float(E_ID), None, Alu.is_equal)
            dmask = consts.tile([BP, S], f32)  # 1 - e: keep original attr cols
            nc.vector.tensor_scalar(dmask, emask, -1.0, 1.0, Alu.mult, Alu.add)
            idx1 = consts.tile([BP, S], f32)  # 1..S per partition
            nc.gpsimd.iota(idx1, pattern=[[1, S]], base=1, channel_multiplier=0,
                           allow_small_or_imprecise_dtypes=True)
            bidx1 = consts.tile([BP, S], f32)  # j+1 at B positions else 0
            nc.vector.tensor_mul(bidx1, idx1, isb)
            lastb1 = consts.tile([BP, S], f32)  # (last B pos)+1, 0 if none
            nc.vector.tensor_tensor_scan(lastb1, bidx1, bidx1, 0.0, Alu.max, Alu.max)
            start_t = consts.tile([BP, S], f32)  # segment start = max(lastb-1, 0)
            nc.vector.tensor_scalar(start_t, lastb1, -1.0, 0.0, Alu.add, Alu.max)
            cnt = consts.tile([BP, S], f32)  # j - start + 1
            nc.vector.tensor_sub(cnt, idx1, start_t)
            invc = consts.tile([BP, S], f32)
            nc.vector.reciprocal(invc, cnt)
            icm = consts.tile([BP, S], f32)   # e/count: mean scale at E, 0 elsewhere
            nc.vector.tensor_mul(icm, invc, emask)
            # logits_attr[k,t] = u_t*(W.T@seg)[k,t] - v_t*(W.T@seg)[k,t-1]
            # with u = (1-e) + e/count, v = r*(1-e)  (x_t = seg_t - r_t*seg_{t-1})
            urow = consts.tile([BP, S], f32)
            nc.vector.tensor_add(urow, dmask, icm)
            rmask8 = consts.tile([BP, S], f32)
            nc.vector.tensor_scalar(rmask8, lab, float(B_ID), None, Alu.not_equal)
            vrow = consts.tile([BP, S], f32)
            nc.vector.tensor_mul(vrow, rmask8, dmask)

            # rB broadcasts gate the first scans: build them on the PE
            # (ones-column outer product), which is idle early -- the DMA
            # path would queue behind the streaming input loads.
            rBs = []
            for b in range(BP):
                rm1 = bcp2.tile([1, S], f32r, tag="rm1")  # row's reset mask at partition 0
                nc.vector.tensor_scalar(rm1, lab1[0:1, b * S:(b + 1) * S], float(B_ID), None, Alu.not_equal)
                rb_ps = tpps.tile([128, S], f32, tag="tp")
                nc.tensor.matmul(rb_ps, ones_r, rm1, start=True, stop=True)
                rB = bcp.tile([128, S], f32r, tag="rB")
                nc.scalar.copy(rB, rb_ps)
                rBs.append(rB)
            nc.gpsimd.dma_start(out=rows[0], in_=urow)
            nc.gpsimd.dma_start(out=rows[1], in_=vrow)
            bcasts = []
            for b in range(BP):
                uB = bcp2.tile([NA, S], f32, tag="uB")
                nc.gpsimd.dma_start(out=uB, in_=rows[0, b:b + 1, :].to_broadcast([NA, S]))
                vB = bcp2.tile([NA, S], f32, tag="vB")
                nc.gpsimd.dma_start(out=vB, in_=rows[1, b:b + 1, :].to_broadcast([NA, S]))
                bcasts.append((rBs[b], uB, vB))

            for b in range(BP):
                rB, uB, vB = bcasts[b]

                attr_nat, span_nat = [], []
                for t in range(TC):
                    a = natp.tile([128, H], f32r, tag="attr_nat")
                    nc.sync.dma_start(out=a, in_=attr_h[b, t * 128:(t + 1) * 128, :])
                    attr_nat.append(a)
                    s = natp.tile([128, H], f32r, tag="span_nat")
                    nc.sync.dma_start(out=s, in_=span_h[b, t * 128:(t + 1) * 128, :])
                    span_nat.append(s)

                l_span = lsp1.tile([NS, S], f32, tag="lspan")
                l_attr2 = lsp1.tile([NA, S], f32, tag="lattr2")  # W.T @ segsums

                for c in range(HC):
                    hs = slice(c * 128, (c + 1) * 128)
                    # attr transposes first: they gate the scan (the DVE is the
                    # pacing engine), span transposes only gate an ACT copy.
                    aT_ps = tpps.tile([128, S], f32r, tag="tp")
                    for t in range(TC):
                        nc.tensor.transpose(aT_ps[:, t * 128:(t + 1) * 128],
                                            attr_nat[t][:, hs], ident)

                    # segmented cumsum along tokens: state = r*state + x
                    seg = tsb.tile([128, S], f32r, tag="seg")
                    nc.vector.tensor_tensor_scan(seg, rB, aT_ps, 0.0, Alu.mult, Alu.add)

                    sT_ps = tpps.tile([128, S], f32r, tag="tp")
                    for t in range(TC):
                        nc.tensor.transpose(sT_ps[:, t * 128:(t + 1) * 128],
                                            span_nat[t][:, hs], ident)
                    sT_sb = tsb.tile([128, S], f32r, tag="sT")
                    nc.scalar.copy(sT_sb, sT_ps)

                    nc.tensor.matmul(l_attr2, w_attr_sb[:, c, :], seg,
                                     start=(c == 0), stop=(c == HC - 1))
                    nc.tensor.matmul(l_span, w_span_sb[:, c, :], sT_sb,
                                     start=(c == 0), stop=(c == HC - 1))

                oS = outp.tile([NS, S], f32, tag="oS")
                nc.scalar.activation(oS, l_span, mybir.ActivationFunctionType.Identity,
                                     bias=bs_sb, scale=1.0)
                nc.gpsimd.dma_start(out=out_spanT[b], in_=oS)
                m1 = outp.tile([NA, S], f32, tag="m1")
                nc.vector.tensor_mul(m1, l_attr2, uB)
                m2 = outp.tile([NA, S], f32, tag="m2")
                nc.vector.tensor_mul(m2[:, 1:S], l_attr2[:, 0:S - 1], vB[:, 1:S])
                nc.vector.tensor_sub(m1[:, 1:S], m1[:, 1:S], m2[:, 1:S])
                oA = outp.tile([NA, S], f32, tag="oA")
                nc.scalar.activation(oA, m1, mybir.ActivationFunctionType.Identity,
                                     bias=ba_sb, scale=1.0)
                nc.gpsimd.dma_start(out=out_attrT[b], in_=oA)

    nc.compile()
    _PROGRAM_CACHE["nc"] = nc
    return nc


def make_in_maps(span_labels, span_hiddens, attr_hiddens, W_span, b_span, W_attr, b_attr):
    labels_f = np.asarray(span_labels).astype(np.float32)
    span_f = np.ascontiguousarray(np.asarray(span_hiddens, dtype=np.float32))
    attr_f = np.ascontiguousarray(np.asarray(attr_hiddens, dtype=np.float32))
    ws = np.ascontiguousarray(np.asarray(W_span, dtype=np.float32))
    bs = np.ascontiguousarray(np.asarray(b_span, dtype=np.float32))
    wa = np.ascontiguousarray(np.asarray(W_attr, dtype=np.float32))
    ba = np.ascontiguousarray(np.asarray(b_attr, dtype=np.float32))
    in_maps = []
    for i in range(NCORES):
        sl = slice(i * BP, (i + 1) * BP)
        in_maps.append({
            "labels": np.ascontiguousarray(labels_f[sl]),
            "span_h": np.ascontiguousarray(span_f[sl]),
            "attr_h": np.ascontiguousarray(attr_f[sl]),
            "w_span": ws, "b_span": bs, "w_attr": wa, "b_attr": ba,
        })
    return in_maps


def assemble(results):
    lspanT = np.stack([r["out_spanT"] for r in results])  # [NCORES, BP, NS, S]
    lattrT = np.stack([r["out_attrT"] for r in results])
    logits_span = lspanT.reshape(B, NS, S).transpose(0, 2, 1).copy()
    logits_attr = lattrT.reshape(B, NA, S).transpose(0, 2, 1).copy()
    return logits_span, logits_attr


def run(inputs, trace=False, tmpdir=None):
    """Run on the 8 NeuronCores. Returns ((logits_span, logits_attr), exec_time_ns)."""
    from concourse.bass_utils import run_bass_kernel_spmd

    nc = build_program()
    in_maps = make_in_maps(**{k: inputs[k] for k in (
        "span_labels", "span_hiddens", "attr_hiddens",
        "W_span", "b_span", "W_attr", "b_attr")})
    if trace:
        _register_ntff_hook()
    res = run_bass_kernel_spmd(nc, in_maps, list(range(NCORES)), trace=trace,
                               tmpdir=tmpdir)
    return assemble(res.results), res.exec_time_ns


def _register_ntff_hook():
    """The agent image lacks antenv.axon_hooks; synthesize it so trace=True
    can reach the axon NTFF profiler. No-op if already registered."""
    import sys, types
    try:
        import antenv.axon_hooks  # noqa: F401
        return
    except ImportError:
        pass
    mod = types.ModuleType("antenv.axon_hooks")
    _h = [None]
    mod.set_axon_ntff_profile_hook = lambda h: _h.__setitem__(0, h)
    mod.get_axon_ntff_profile_hook = lambda: _h[0]
    sys.modules["antenv.axon_hooks"] = mod
    try:
        from trn_agent_boot.trn_boot import _ntff_profile_via_ctypes
        mod.set_axon_ntff_profile_hook(_ntff_profile_via_ctypes("/opt/axon/libaxon_pjrt.so"))
    except Exception:
        pass
    import concourse.bass_utils as bass_utils
    bass_utils.upload_artifacts = lambda tmpdir: "local://" + str(tmpdir)


def kernel(span_labels, span_hiddens, attr_hiddens, W_span, b_span, W_attr, b_attr):
    (logits_span, logits_attr), _ = run({
        "span_labels": span_labels, "span_hiddens": span_hiddens,
        "attr_hiddens": attr_hiddens, "W_span": W_span, "b_span": b_span,
        "W_attr": W_attr, "b_attr": b_attr,
    })
    return logits_span, logits_attr
